# revision 12
# baseline (speedup 1.0000x reference)
"""MultiHeadAttention on 8 TRN2 NeuronCores — v4 (batch-split).

Sharding: 2 groups of 4 cores. Group g owns batch g; core (g, l) owns
heads 4l..4l+4 of batch g and token shard 512l..512(l+1) of that batch.

vs v3 (head-parallel over 8 cores, both batches per core):
- AllGather of x runs within a 4-core group (2MB out vs 4MB) and phase 1
  starts on the core's LOCAL token tile before the gather lands.
- ReduceScatter is [2048,1024] f32 within the group (8MB vs 16MB), split
  into two token-halves so the first RS overlaps the second half of
  phase 3.
- Phase-3 PSUM drains moved from ACT to DVE (ACT is the global
  bottleneck: softmax exp).
Per-core compute (QKV, attention, out-proj FLOPs) is unchanged.
"""

import numpy as np
import ml_dtypes

import concourse.bass as bass
import concourse.tile as tile
from concourse import bacc, mybir
from concourse.bass_utils import run_bass_kernel_spmd

N_CORES = 8
B, S, D = 2, 2048, 1024
TOK = B * S  # 4096
GRP = 4  # cores per group (one batch per group)
ST = S // GRP  # 512 tokens per core shard
F32 = mybir.dt.float32
F32R = mybir.dt.float32r
BF16 = mybir.dt.bfloat16
Exp = mybir.ActivationFunctionType.Exp
Identity = mybir.ActivationFunctionType.Identity
BF = ml_dtypes.bfloat16

_cache = {}


def _build(repeat=1, phases=("ag", "p1", "p2", "p3", "rs")):
    nc = bacc.Bacc("TRN2", target_bir_lowering=False, debug=False,
                   num_devices=N_CORES)
    xp_d = nc.dram_tensor("xp", [128, 8, 512], BF16, kind="ExternalInput").ap()
    wq_d = nc.dram_tensor("wqp", [128, 8, 2, 128], BF16, kind="ExternalInput").ap()
    wk_d = nc.dram_tensor("wkp", [128, 8, 2, 128], BF16, kind="ExternalInput").ap()
    wv_d = nc.dram_tensor("wvp", [128, 8, 2, 128], BF16, kind="ExternalInput").ap()
    wo_d = nc.dram_tensor("wos", [128, 2, D], BF16, kind="ExternalInput").ap()
    bq_d = nc.dram_tensor("bqc", [128, 2], F32, kind="ExternalInput").ap()
    bk_d = nc.dram_tensor("bkc", [128, 2], F32, kind="ExternalInput").ap()
    bv_d = nc.dram_tensor("bvc", [128, 2], F32, kind="ExternalInput").ap()
    id_d = nc.dram_tensor("ident", [128, 128], F32, kind="ExternalInput").ap()
    vo_d = nc.dram_tensor("vones", [128, 16, 1], F32, kind="ExternalInput").ap()
    out_d = nc.dram_tensor("out", [512, D], F32, kind="ExternalOutput").ap()

    with tile.TileContext(nc) as tc:
        with (
            tc.tile_pool(name="dram", bufs=1, space="DRAM") as dram,
            tc.tile_pool(name="persist", bufs=1) as pp,
        ):
            xb_bo = dram.tile([128, 8, 512], BF16, tag="xbo")
            gath_x = dram.tile([4, 128, 8, 512], BF16, tag="gx")
            part_d = dram.tile([S, D], F32, tag="part")
            rs_d = dram.tile([512, D], F32, tag="rsd")

            wq_sb = pp.tile([128, 8, 2, 128], BF16, tag="wq")
            wk_sb = pp.tile([128, 8, 2, 128], BF16, tag="wk")
            wv_sb = pp.tile([128, 8, 2, 128], BF16, tag="wv")
            wo_sb = pp.tile([128, 2, D], BF16, tag="wo")
            nc.gpsimd.dma_start(wq_sb[:], wq_d[:])
            nc.gpsimd.dma_start(wk_sb[:], wk_d[:])
            nc.gpsimd.dma_start(wv_sb[:], wv_d[:])
            nc.gpsimd.dma_start(wo_sb[:], wo_d[:])
            bq_sb = pp.tile([128, 2], F32, tag="bq")
            bk_sb = pp.tile([128, 2], F32, tag="bk")
            bv_sb = pp.tile([128, 2], F32, tag="bv")
            id_sb = pp.tile([128, 128], F32R, tag="iden")
            nc.gpsimd.dma_start(bq_sb[:], bq_d[:])
            nc.gpsimd.dma_start(bk_sb[:], bk_d[:])
            nc.gpsimd.dma_start(bv_sb[:], bv_d[:])
            nc.gpsimd.dma_start(id_sb[:], id_d[:].bitcast(F32R))

            QT = pp.tile([128, 2, S], F32R, tag="QT")
            KT = pp.tile([128, 2, S], F32R, tag="KT")
            VT = pp.tile([128, 2, S], F32R, tag="VT")
            Vbig = pp.tile([128, 4, 16, 65], F32R, tag="vbig")
            for p in range(4):
                nc.gpsimd.dma_start(Vbig[:, p, :, 64:65], vo_d[:].bitcast(F32R))
            OTb = pp.tile([128, 2, S], BF16, tag="otb")

            ones_f = pp.tile([128, 64], F32, tag="onesf")
            nc.vector.memset(ones_f[:], 1.0)
            onesr = pp.tile([128, 64], F32R, tag="onesr")
            nc.vector.tensor_copy(onesr[:], ones_f[:])

            if phases != ("ag", "p1", "p2", "p3", "rs"):
                # attribution runs: init tiles that skipped phases would
                # leave unwritten
                for t in (QT, KT, VT, Vbig):
                    nc.vector.memset(t[:].bitcast(F32), 0.01)
                nc.vector.memset(OTb[:].bitcast(F32), 0.01)
            for _ in range(repeat):
                _body(nc, tc, xp_d, out_d, xb_bo, gath_x, part_d, rs_d,
                      (wq_sb, wk_sb, wv_sb), (bq_sb, bk_sb, bv_sb),
                      wo_sb, id_sb, onesr, QT, KT, VT, Vbig, OTb, phases)
    nc.compile()
    return nc


def _body(nc, tc, xp_d, out_d, xb_bo, gath_x, part_d, rs_d,
          w_sbs, b_sbs, wo_sb, id_sb, onesr, QT, KT, VT, Vbig, OTb,
          phases=("ag", "p1", "p2", "p3", "rs")):
    PSUM = bass.MemorySpace.PSUM
    groups = [[0, 1, 2, 3], [4, 5, 6, 7]]
    wq_sb, wk_sb, wv_sb = w_sbs
    bq_sb, bk_sb, bv_sb = b_sbs
    pid = None  # tiles chosen per-lane would need runtime id; use all-gather

    # ---- AllGather x within the 4-core group ----
    if "ag" in phases:
        nc.gpsimd.dma_start(xb_bo[:], xp_d[:])
        nc.gpsimd.collective_compute(
            "AllGather", mybir.AluOpType.bypass, replica_groups=groups,
            ins=[xb_bo[:].opt()], outs=[gath_x[:].opt()])

    # ---- Phase 1: Q/K/V [dk, tok] chains; V transposed to [tok, dk] ----
    if "p1" in phases:
        with (
            tc.tile_pool(name="xt", bufs=2) as xtp,
            tc.tile_pool(name="qkpsum", bufs=2, space=PSUM) as qkp,
            tc.tile_pool(name="tpsum", bufs=2, space=PSUM) as tpp,
        ):
            for tt in range(4):  # 512-token tiles of my batch
                xb = xtp.tile([128, 8, 512], BF16, tag="xb", name="xb")
                nc.gpsimd.dma_start(xb[:], gath_x[tt, :, :, :])
                for w, (wsb, bsb, dst) in enumerate(
                        ((wq_sb, bq_sb, QT), (wk_sb, bk_sb, KT),
                         (wv_sb, bv_sb, VT))):
                    for h in range(2):
                        acc = qkp.tile([128, 512], F32, tag=f"acc{w}",
                                       name=f"acc{w}{h}")
                        for j in range(8):
                            nc.tensor.matmul(acc[:], wsb[:, j, h, :],
                                             xb[:, j, :],
                                             start=(j == 0), stop=(j == 7))
                        nc.scalar.activation(
                            dst[:, h, 512 * tt:512 * (tt + 1)], acc[:],
                            Identity, bias=bsb[:, h:h + 1], scale=1.0)
                for hp in range(2):
                    for tb in range(4):
                        t0 = 512 * tt + 128 * tb
                        kc = 4 * tt + tb
                        tps = tpp.tile([128, 128], F32R, tag="tps",
                                       name="tps")
                        nc.tensor.transpose(tps[:], VT[:, hp, t0:t0 + 128],
                                            id_sb[:])
                        nc.vector.tensor_copy(
                            Vbig[:, 2 * hp:2 * hp + 2, kc, 0:64], tps[:])

    # ---- Phase 2: attention per head ----
    # Software-pipelined: QK for chunk-group g+1 is emitted BEFORE exp/AV
    # of group g, so the PE's in-order queue never stalls on ACT. A
    # chunk-group is one s_ps tile = 2 k-chunks of 128 tokens x 512 q.
    if "p2" in phases:
        with (
            tc.tile_pool(name="pt", bufs=3) as ptp,
            tc.tile_pool(name="spsum", bufs=3, space=PSUM) as sp,
            tc.tile_pool(name="opsum", bufs=1, space=PSUM) as op,
            tc.tile_pool(name="bpsum", bufs=1, space=PSUM) as bp,
            tc.tile_pool(name="nrm", bufs=2) as nrm,
        ):
            NG = 8  # chunk-groups per (p, qt)
            steps = []  # flattened (p, qt, g) sequence
            for p in range(4):
                for qt in range(4):
                    for g in range(NG):
                        steps.append((p, qt, g))

            s_tiles = {}
            o_tiles = {}

            def emit_qk(p, qt, g):
                hp, hh = divmod(p, 2)
                KT_h = KT[64 * hh:64 * (hh + 1), hp, :]
                QT_h = QT[64 * hh:64 * (hh + 1), hp, :]
                q0 = 512 * qt
                if g == 0:
                    o_tiles[(p, qt)] = op.tile([65, 512], F32, tag="oacc",
                                               name=f"oa{p}{qt}")
                s_ps = sp.tile([128, 1024], F32, tag="sps", name=f"s{p}{qt}{g}")
                s_tiles[(p, qt, g)] = s_ps
                for u in range(2):
                    k0 = 128 * (2 * g + u)
                    nc.tensor.matmul(s_ps[:, 512 * u:512 * (u + 1)],
                                     KT_h[:, k0:k0 + 128],
                                     QT_h[:, q0:q0 + 512],
                                     start=True, stop=True)

            def emit_expav(p, qt, g):
                s_ps = s_tiles.pop((p, qt, g))
                o_acc = o_tiles[(p, qt)]
                pt_t = ptp.tile([128, 1024], F32R, tag="pt", name=f"pt{p}{qt}{g}")
                nc.scalar.activation(pt_t[:], s_ps[:], Exp,
                                     bias=0.0, scale=0.125)
                for u in range(2):
                    kc = 2 * g + u
                    nc.tensor.matmul(o_acc[:], Vbig[:, p, kc, :],
                                     pt_t[:, 512 * u:512 * (u + 1)],
                                     start=(kc == 0), stop=(kc == 2 * NG - 1))
                if g == NG - 1:
                    hp, hh = divmod(p, 2)
                    q0 = 512 * qt
                    r_f = nrm.tile([128, 512], F32, tag="rf", name="r_f")
                    nc.vector.reciprocal(r_f[64:65, :], o_acc[64:65, :])
                    r_t = nrm.tile([128, 512], F32R, tag="rt", name="r_t")
                    nc.vector.tensor_copy(r_t[64:65, :], r_f[64:65, :])
                    bc_ps = bp.tile([64, 512], F32, tag="bc", name="bc_ps")
                    nc.tensor.matmul(bc_ps[:], onesr[64:65, :],
                                     r_t[64:65, :], start=True, stop=True)
                    bc_sb = nrm.tile([64, 512], F32, tag="bcs", name="bc_sb")
                    nc.vector.tensor_copy(bc_sb[:], bc_ps[:])
                    nc.vector.tensor_mul(
                        OTb[64 * hh:64 * (hh + 1), hp, q0:q0 + 512],
                        o_acc[0:64, :], bc_sb[:])
                    del o_tiles[(p, qt)]

            LOOKAHEAD = 2
            for i in range(len(steps) + LOOKAHEAD):
                if i < len(steps):
                    emit_qk(*steps[i])
                if i >= LOOKAHEAD:
                    emit_expav(*steps[i - LOOKAHEAD])

    # ---- Phase 3: partial out = O^T @ wo_slice ; chunked ReduceScatter ----
    if "p3" in phases:
        with (
            tc.tile_pool(name="fout", bufs=2) as fo,
            tc.tile_pool(name="fpsum", bufs=2, space=PSUM) as fp,
        ):
            for half in range(2):
                for tb in range(8):
                    t0 = 1024 * half + 128 * tb
                    acc = fp.tile([128, 1024], F32, tag="facc", name="acc")
                    for h in range(2):
                        for ns in range(2):
                            n0 = 512 * ns
                            nc.tensor.matmul(acc[:, n0:n0 + 512],
                                             OTb[:, h, t0:t0 + 128],
                                             wo_sb[:, h, n0:n0 + 512],
                                             start=(h == 0), stop=(h == 1))
                    o_sb = fo.tile([128, 1024], F32, tag="fo", name="o_sb")
                    nc.vector.tensor_copy(o_sb[:], acc[:])
                    nc.gpsimd.dma_start(part_d[t0:t0 + 128, :], o_sb[:])
                if "rs" in phases:
                    h0 = 1024 * half
                    nc.gpsimd.collective_compute(
                        "ReduceScatter", mybir.AluOpType.add,
                        replica_groups=groups,
                        ins=[part_d[h0:h0 + 1024, :].opt()],
                        outs=[rs_d[256 * half:256 * (half + 1), :].opt()])
            if "rs" in phases:
                nc.gpsimd.dma_start(out_d[:], rs_d[:])


def _in_maps(x, wq, bq, wk, bk, wv, bv, wo, bo):
    xt = x.reshape(TOK, D)

    def pack_x(c):
        xs = xt[512 * c:512 * (c + 1), :]
        return np.ascontiguousarray(
            xs.T.reshape(8, 128, 512).transpose(1, 0, 2)).astype(BF)

    def pack_w(w, c):
        wc = w[:, 256 * c:256 * (c + 1)]
        return np.ascontiguousarray(
            wc.reshape(8, 128, 2, 128).transpose(1, 0, 2, 3)).astype(BF)

    def pack_b(b, c):
        return np.ascontiguousarray(
            b[256 * c:256 * (c + 1)].reshape(2, 128).T)

    vones = np.ones((128, 16, 1), dtype=np.float32)
    ident = np.eye(128, dtype=np.float32)
    maps = []
    for c in range(N_CORES):
        l = c % GRP  # head-slice lane (heads replicated across groups)
        maps.append({
            "xp": pack_x(c),
            "wqp": pack_w(wq, l), "wkp": pack_w(wk, l), "wvp": pack_w(wv, l),
            "wos": np.ascontiguousarray(
                wo[256 * l:256 * (l + 1), :].reshape(2, 128, D)
                .transpose(1, 0, 2)).astype(BF),
            "bqc": pack_b(bq, l), "bkc": pack_b(bk, l), "bvc": pack_b(bv, l),
            "ident": ident,
            "vones": vones,
        })
    return maps


def _assemble(results, bo):
    """Each core (g,l) returns [512, D]: row 256*j + r = token
    2048*g + 1024*j + 256*l + r."""
    out = np.empty((TOK, D), np.float32)
    for c in range(N_CORES):
        g, l = divmod(c, GRP)
        r = results[c]["out"]
        for j in range(2):
            t0 = 2048 * g + 1024 * j + 256 * l
            out[t0:t0 + 256] = r[256 * j:256 * (j + 1)]
    return (out + bo.reshape(1, D)).reshape(B, S, D)


def kernel(**inputs):
    x = np.asarray(inputs["x"], dtype=np.float32)
    bo = np.asarray(inputs["bo"], np.float32)
    maps = _in_maps(
        x,
        np.asarray(inputs["wq"], np.float32), np.asarray(inputs["bq"], np.float32),
        np.asarray(inputs["wk"], np.float32), np.asarray(inputs["bk"], np.float32),
        np.asarray(inputs["wv"], np.float32), np.asarray(inputs["bv"], np.float32),
        np.asarray(inputs["wo"], np.float32), bo,
    )
    if "nc" not in _cache:
        _cache["nc"] = _build()
    res = run_bass_kernel_spmd(_cache["nc"], maps,
                               core_ids=list(range(N_CORES)), trace=False)
    return _assemble(res.results, bo)
